# revision 1
# baseline (speedup 1.0000x reference)
"""GAT (2-layer, 8-head) fused Bass kernel for 8 trn2 NeuronCores.

Sharding: nodes (rows of x) split 512/core. Layer-1 h/s computed locally per
core, AllGather'd (h+ones in bf16, scores in fp32); each core computes its
512xN attention block for all 8 heads; layer-1 output xc (+ its layer-2
projection, ones and scores) AllGather'd again (fp32); each core computes its
512xN layer-2 attention block and the final log_softmax rows.

Key algebra: with s_i = h_i . a_src, d_j = h_j . a_dst,
  exp(leakyrelu(s_i + d_j)) = max(exp(s_i)exp(d_j), exp(.2 s_i)exp(.2 d_j))
and softmax over j is invariant to any per-i scale, so the attention
numerator can be taken as P[i,j] = max(b_j, w_i * dd_j) with
  b_j = exp(d_j), w_i = exp(-0.8 s_i), dd_j = exp(0.2 d_j).
One DVE/GPSIMD tensor_scalar (mult, max) per [128,512] tile; exp only on
vectors. elu(x) = max(x, min(exp(x)-1, 0)); log_softmax via Exp(accum_out)+Ln.
Matmuls run as float32r (1 cyc/row) or bf16; fp32 would be 4 cyc/row.
"""

import numpy as np

N, NFEAT, NHID, NCLASS, NHEADS = 4096, 512, 64, 16, 8
NC = 8                      # cores
NQ = N // NC                # 512 own nodes per core
QT = NQ // 128              # 4 query tiles per core
JT = N // 128               # 32 key tiles
ALPHA = 0.2
HW = NHID * NHEADS          # 512
HXC = NHEADS * (NHID + 1)   # 520: per-head 64 h cols + ones col (bf16 AG1)
AGC2 = 18                   # AG2: 16 outh + 1 ones + 1 sdst2

_CACHE = {}


def _build_nc(no_cc=False, no_l1=False):
    import concourse.bass as bass
    import concourse.bacc as bacc
    import concourse.mybir as mybir
    import concourse.tile as tile
    from concourse.masks import make_identity

    fp32 = mybir.dt.float32
    fp32r = mybir.dt.float32r
    bf16 = mybir.dt.bfloat16
    AX = mybir.AxisListType.X
    OP = mybir.AluOpType
    AF = mybir.ActivationFunctionType

    nc = bacc.Bacc()
    xT = nc.declare_dram_parameter("xT", [NFEAT, NQ], fp32, isOutput=False)
    Whr = nc.declare_dram_parameter("Whr", [NFEAT, HW], fp32, isOutput=False)
    Asd = nc.declare_dram_parameter("Asd", [NFEAT, 16], fp32, isOutput=False)
    Wo = nc.declare_dram_parameter("Wo", [HW, NCLASS], fp32, isOutput=False)
    aod = nc.declare_dram_parameter("aod", [2, NCLASS], fp32, isOutput=False)
    out = nc.declare_dram_parameter("out", [NQ, NCLASS], fp32, isOutput=True)

    with tile.TileContext(nc) as tc:
        with (
            tc.tile_pool(name="const", bufs=1) as constp,
            tc.tile_pool(name="big", bufs=1) as bigp,
            tc.tile_pool(name="work", bufs=3) as workp,
            tc.tile_pool(name="pp", bufs=8) as ppool,
            tc.tile_pool(name="ps_acc", bufs=3, space="PSUM") as ps_acc,
            tc.tile_pool(name="ps_t", bufs=4, space="PSUM") as ps_t,
            tc.tile_pool(name="dram", bufs=1, space="DRAM") as dramp,
        ):
            v, sc, g, te, dma = nc.vector, nc.scalar, nc.gpsimd, nc.tensor, nc.sync

            ident = constp.tile([128, 128], fp32, tag="ident")
            make_identity(nc, ident[:])
            # sel[k, h*128+m] = 1 iff k == h: one-hot row selector for
            # partition-broadcast matmuls (out = sel_h.T @ rows)
            self_f = constp.tile([8, 8 * 128], fp32, tag="self_f")
            g.memset(self_f[:], 0.0)
            g.affine_select(
                out=self_f[:].rearrange("k (h m) -> k h m", m=128),
                in_=self_f[:].rearrange("k (h m) -> k h m", m=128),
                compare_op=mybir.AluOpType.not_equal,
                fill=1.0, base=0, channel_multiplier=1,
                pattern=[[-1, 8], [0, 128]])
            sel = constp.tile([8, 8 * 128], fp32r, tag="sel")
            v.tensor_copy(sel[:], self_f[:])

            # ---- A. load params (fp32 load + fp32r cast for PE use) ----
            xT_sb, whr_sb, asd_sb, wo_sb = [], [], [], []
            for k in range(4):
                tf = workp.tile([128, NQ], fp32, tag="pload", name=f"xTf{k}")
                dma.dma_start(tf[:], xT[k * 128:(k + 1) * 128, :])
                t = constp.tile([128, NQ], fp32r, tag=f"xT{k}", name=f"xT{k}")
                v.tensor_copy(t[:], tf[:])
                xT_sb.append(t)
                tf = workp.tile([128, HW], fp32, tag="pload", name=f"whrf{k}")
                dma.dma_start(tf[:], Whr[k * 128:(k + 1) * 128, :])
                t = constp.tile([128, HW], fp32r, tag=f"whr{k}", name=f"whr{k}")
                v.tensor_copy(t[:], tf[:])
                whr_sb.append(t)
                tf = workp.tile([128, 16], fp32, tag="pload16", name=f"asdf{k}")
                dma.dma_start(tf[:], Asd[k * 128:(k + 1) * 128, :])
                t = constp.tile([128, 16], fp32r, tag=f"asd{k}", name=f"asd{k}")
                v.tensor_copy(t[:], tf[:])
                asd_sb.append(t)
                t = constp.tile([128, 16], fp32, tag=f"wo{k}", name=f"wo{k}")
                dma.dma_start(t[:], Wo[k * 128:(k + 1) * 128, :])
                wo_sb.append(t)
            aos_b = constp.tile([128, 16], fp32, tag="aos_b")
            dma.dma_start(aos_b[:], aod[0:1, :].to_broadcast((128, 16)))
            aod_b = constp.tile([128, 16], fp32, tag="aod_b")
            dma.dma_start(aod_b[:], aod[1:2, :].to_broadcast((128, 16)))

            ag1h_in = dramp.tile([NQ, HXC], bf16, tag="ag1h_in")
            ag1h_out = dramp.tile([N, HXC], bf16, tag="ag1h_out",
                                  addr_space="Local" if no_cc else "Shared")
            ag1s_in = dramp.tile([NQ, 16], fp32, tag="ag1s_in")
            ag1s_out = dramp.tile([N, 16], fp32, tag="ag1s_out",
                                  addr_space="Local" if no_cc else "Shared")
            ag2_in = dramp.tile([NQ, AGC2], fp32, tag="ag2_in")
            ag2_out = dramp.tile([N, AGC2], fp32, tag="ag2_out",
                                 addr_space="Local" if no_cc else "Shared")

            # ---- B. h_ownT (feat-major) ----
            hT_sb = []
            for f in range(4):
                ps = ps_acc.tile([128, NQ], fp32, tag="acc")
                for k in range(4):
                    te.matmul(ps[:], whr_sb[k][:, f * 128:(f + 1) * 128],
                              xT_sb[k][:], start=(k == 0), stop=(k == 3))
                t = constp.tile([128, NQ], fp32r, tag=f"hT{f}", name=f"hT{f}")
                (v.tensor_copy if f % 2 else sc.copy)(t[:], ps[:])
                hT_sb.append(t)

            # ---- D. s_own [16, NQ] rows 0:8 src, 8:16 dst ----
            s_ps = ps_acc.tile([16, NQ], fp32, tag="acc")
            for k in range(4):
                te.matmul(s_ps[:], asd_sb[k][:], hT_sb[k][:],
                          start=(k == 0), stop=(k == 3))
            s_sb = constp.tile([16, NQ], fp32, tag="s_sb")
            v.tensor_copy(s_sb[:], s_ps[:])

            # ---- F. w panel: exp(-0.8 * s_src), bcast via K=1 matmul ----
            w_sb = constp.tile([NHEADS, NQ], fp32r, tag="w_sb")
            sc.activation(w_sb[:], s_sb[0:NHEADS, :], AF.Exp, scale=-0.8)
            wb = []
            for h in range(NHEADS):
                bp = ps_t.tile([128, NQ], fp32, tag="bigtp", bufs=1)
                te.matmul(bp[:], sel[:, h * 128:(h + 1) * 128],
                          w_sb[:], start=True, stop=True)
                t = constp.tile([128, NQ], bf16, tag=f"wb{h}", name=f"wb{h}")
                (v.tensor_copy if h % 2 else sc.copy)(t[:], bp[:])
                wb.append(t)

            # ---- C/E. h_own + stage AG1 (h bf16 + s fp32) ----
            for qt in range(QT):
                ps = ps_acc.tile([128, HW], fp32, tag="acc")
                for k in range(4):
                    te.matmul(ps[:], xT_sb[k][:, qt * 128:(qt + 1) * 128],
                              whr_sb[k][:], start=(k == 0), stop=(k == 3))
                stg = workp.tile([128, HXC], bf16, tag="stage")
                sc.copy(stg[:].rearrange("p (h c) -> p h c", c=65)[:, :, 0:64],
                        ps[:].rearrange("p (h c) -> p h c", c=64))
                g.memset(
                    stg[:].rearrange("p (h c) -> p h c", c=65)[:, :, 64:65], 1.0)
                dma.dma_start(ag1h_in[qt * 128:(qt + 1) * 128, :], stg[:])
                tp = ps_t.tile([128, 16], fp32, tag="tp")
                te.transpose(tp[:], s_sb[:, qt * 128:(qt + 1) * 128],
                             ident[0:16, 0:16])
                stgs = workp.tile([128, 16], fp32, tag="stgs")
                v.tensor_copy(stgs[:], tp[:])
                dma.dma_start(ag1s_in[qt * 128:(qt + 1) * 128, :], stgs[:])

            # ---- G. AllGather 1 (both buffers in one op) ----
            if no_cc:
                for r in range(NC):
                    dma.dma_start(ag1h_out[r * NQ:(r + 1) * NQ, :], ag1h_in[:])
                    dma.dma_start(ag1s_out[r * NQ:(r + 1) * NQ, :], ag1s_in[:])
            else:
                g.collective_compute(
                    "AllGather", OP.bypass,
                    ins=[ag1s_in.opt()], outs=[ag1s_out.opt()],
                    replica_groups=[list(range(NC))],
                )
                g.collective_compute(
                    "AllGather", OP.bypass,
                    ins=[ag1h_in.opt()], outs=[ag1h_out.opt()],
                    replica_groups=[list(range(NC))],
                )

            # ---- H. key-side score panels (bf16) ----
            sd_pan = constp.tile([128, JT * NHEADS], fp32, tag="sd_pan")
            dma.dma_start(
                sd_pan[:].rearrange("p (t h) -> p t h", h=NHEADS),
                ag1s_out[:, 8:16].rearrange("(t p) h -> p t h", p=128))
            b_all = constp.tile([128, JT * NHEADS], fp32, tag="b_all")
            sc.activation(b_all[:], sd_pan[:], AF.Exp)
            d_all = constp.tile([128, JT * NHEADS], fp32, tag="d_all")
            sc.activation(d_all[:], sd_pan[:], AF.Exp, scale=ALPHA)

            # ---- I. hx tiles (persistent keys, bf16) ----
            hx = []
            for jt in range(JT):
                t = bigp.tile([128, HXC], bf16, tag=f"hx{jt}", name=f"hx{jt}")
                dma.dma_start(t[:], ag1h_out[jt * 128:(jt + 1) * 128, :])
                hx.append(t)

            # ---- J/K. layer-1 attention ----
            xr = [bigp.tile([128, HW], fp32, tag=f"xr{qt}", name=f"xr{qt}")
                  for qt in range(QT)]
            xc_sb = [bigp.tile([128, HW], fp32, tag=f"xc{qt}", name=f"xc{qt}")
                     for qt in range(QT)]
            xcT_sb = [constp.tile([128, NQ], fp32, tag=f"xcT{f}", name=f"xcT{f}")
                      for f in range(4)]

            def elu_block(qt, fb):
                # elu on xr cols of head pair fb -> xc_sb, then transpose
                # into xcT_sb[fb] (overlaps with later heads' attention)
                c0, c1 = fb * 128, (fb + 1) * 128
                ex = workp.tile([128, 128], fp32, tag="ex")
                sc.activation(ex[:], xr[qt][:, c0:c1], AF.Exp)
                v.tensor_scalar(ex[:], ex[:], 1.0, 0.0, OP.subtract, OP.min)
                v.tensor_tensor(xc_sb[qt][:, c0:c1], xr[qt][:, c0:c1], ex[:],
                                OP.max)
                tp = ps_t.tile([128, 128], fp32, tag="tp")
                te.transpose(tp[:], xc_sb[qt][:, c0:c1], ident[:])
                eng_copy = sc.copy if fb % 2 else v.tensor_copy
                eng_copy(xcT_sb[fb][:, qt * 128:(qt + 1) * 128], tp[:])

            for h in range(NHEADS if not no_l1 else 0):
                acc = ps_acc.tile([65, NQ], fp32, tag="acc")
                for jt in range(JT):
                    pt = ppool.tile([128, NQ], bf16, tag="pt")
                    eng = g if (jt % 6 == 5) else v
                    eng.tensor_scalar(
                        pt[:], wb[h][:],
                        d_all[:, jt * NHEADS + h:jt * NHEADS + h + 1],
                        b_all[:, jt * NHEADS + h:jt * NHEADS + h + 1],
                        OP.mult, OP.max)
                    te.matmul(acc[:], hx[jt][:, h * 65:(h + 1) * 65], pt[:],
                              start=(jt == 0), stop=(jt == JT - 1))
                fT = workp.tile([65, NQ], fp32, tag="fT")
                sc.copy(fT[:], acc[:])
                den = workp.tile([128, QT], fp32, tag="den")
                tps = []
                for qt in range(QT):
                    tp = ps_t.tile([128, 65], fp32, tag="tp", name=f"tp{qt}")
                    te.transpose(tp[:], fT[:, qt * 128:(qt + 1) * 128],
                                 ident[0:65, 0:65])
                    sc.copy(den[:, qt:qt + 1], tp[:, 64:65])
                    tps.append(tp)
                r = workp.tile([128, QT], fp32, tag="recip")
                v.reciprocal(r[:], den[:])
                for qt in range(QT):
                    v.tensor_scalar(xr[qt][:, h * 64:(h + 1) * 64],
                                    tps[qt][:, 0:64], r[:, qt:qt + 1], None,
                                    OP.mult)
                if h % 2 == 1:
                    for qt in range(QT):
                        elu_block(qt, h // 2)

            # ---- K2/L fallback for no_l1 timing variant ----
            w2tmp = constp.tile([128, QT], fp32, tag="w2tmp")
            if no_l1:
                for qt in range(QT):
                    g.memset(xr[qt][:], 0.5)
                for qt in range(QT):
                    for fb in range(4):
                        elu_block(qt, fb)
            stg2s = [bigp.tile([128, AGC2], fp32, tag=f"stage2_{qt}",
                               name=f"stage2_{qt}") for qt in range(QT)]

            # ---- M. outh_own; scores; stage AG2 ----
            for qt in range(QT):
                ps = ps_t.tile([128, 16], fp32, tag="tp")
                for k in range(4):
                    te.matmul(ps[:], xcT_sb[k][:, qt * 128:(qt + 1) * 128],
                              wo_sb[k][:], start=(k == 0), stop=(k == 3))
                stg = stg2s[qt]
                v.tensor_copy(stg[:, 0:16], ps[:])
                g.memset(stg[:, 16:17], 1.0)
                tmp = workp.tile([128, 16], fp32, tag="sdtmp")
                v.tensor_tensor(tmp[:], ps[:], aod_b[:], OP.mult)
                v.tensor_reduce(stg[:, 17:18], tmp[:], AX, OP.add)
                v.tensor_tensor(tmp[:], ps[:], aos_b[:], OP.mult)
                v.tensor_reduce(w2tmp[:, qt:qt + 1], tmp[:], AX, OP.add)
                dma.dma_start(ag2_in[qt * 128:(qt + 1) * 128, :], stg[:])

            # ---- N. w2 bcast: [128,QT] -> row [1,NQ] -> bcast matmul ----
            w2e = constp.tile([128, QT], fp32, tag="w2e")
            sc.activation(w2e[:], w2tmp[:], AF.Exp, scale=-0.8)
            w2tp = ps_t.tile([QT, 128], fp32, tag="tp")
            te.transpose(w2tp[:], w2e[:], ident[:])
            w2tps = constp.tile([QT, 128], fp32r, tag="w2tps")
            v.tensor_copy(w2tps[:], w2tp[:])
            w2b = constp.tile([128, NQ], bf16, tag="w2b")
            for qt in range(QT):
                w2ps = ps_t.tile([128, 128], fp32, tag="tp")
                te.matmul(w2ps[:], sel[0:QT, qt * 128:(qt + 1) * 128],
                          w2tps[:], start=True, stop=True)
                sc.copy(w2b[:, qt * 128:(qt + 1) * 128], w2ps[:])

            # ---- O. AllGather 2 ----
            if no_cc:
                for r in range(NC):
                    dma.dma_start(ag2_out[r * NQ:(r + 1) * NQ, :], ag2_in[:])
            else:
                g.collective_compute(
                    "AllGather", OP.bypass,
                    ins=[ag2_in.opt()], outs=[ag2_out.opt()],
                    replica_groups=[list(range(NC))],
                )

            # ---- P. layer-2 panels ----
            hx2f = constp.tile([128, JT * 17], fp32, tag="hx2f")
            dma.dma_start(
                hx2f[:].rearrange("p (t c) -> p t c", c=17),
                ag2_out[:, 0:17].rearrange("(t p) c -> p t c", p=128))
            hx2 = constp.tile([128, JT * 17], bf16, tag="hx2")
            sc.copy(hx2[:], hx2f[:])
            sd2 = constp.tile([128, JT], fp32, tag="sd2")
            dma.dma_start(
                sd2[:].rearrange("p (t c) -> p t c", c=1),
                ag2_out[:, 17:18].rearrange("(t p) c -> p t c", p=128))
            b2 = constp.tile([128, JT], fp32, tag="b2")
            sc.activation(b2[:], sd2[:], AF.Exp)
            d2 = constp.tile([128, JT], fp32, tag="d2")
            sc.activation(d2[:], sd2[:], AF.Exp, scale=ALPHA)

            # ---- Q. layer-2 attention ----
            acc2 = ps_acc.tile([17, NQ], fp32, tag="acc")
            for jt in range(JT):
                pt = ppool.tile([128, NQ], bf16, tag="pt")
                eng = g if (jt % 6 == 5) else v
                eng.tensor_scalar(pt[:], w2b[:],
                                  d2[:, jt:jt + 1], b2[:, jt:jt + 1],
                                  OP.mult, OP.max)
                te.matmul(acc2[:], hx2[:, jt * 17:(jt + 1) * 17], pt[:],
                          start=(jt == 0), stop=(jt == JT - 1))
            f2 = workp.tile([17, NQ], fp32, tag="f2")
            sc.copy(f2[:], acc2[:])

            # ---- R. normalize, elu, log_softmax, store (ACT batched) ----
            den2 = workp.tile([128, QT], fp32, tag="den")
            t2s = []
            for qt in range(QT):
                tp = ps_t.tile([128, 17], fp32, tag="tp", name=f"t2_{qt}")
                te.transpose(tp[:], f2[:, qt * 128:(qt + 1) * 128],
                             ident[0:17, 0:17])
                sc.copy(den2[:, qt:qt + 1], tp[:, 16:17])
                t2s.append(tp)
            r2 = workp.tile([128, QT], fp32, tag="recip")
            v.reciprocal(r2[:], den2[:])
            os_, eos, elus, ses = [], [], [], []
            for qt in range(QT):
                o = workp.tile([128, NCLASS], fp32, tag=f"o{qt}", name=f"o{qt}")
                v.tensor_scalar(o[:], t2s[qt][:, 0:16], r2[:, qt:qt + 1], None,
                                OP.mult)
                os_.append(o)
            for qt in range(QT):
                eo = workp.tile([128, NCLASS], fp32, tag=f"eo{qt}",
                                name=f"eo{qt}")
                sc.activation(eo[:], os_[qt][:], AF.Exp)
                eos.append(eo)
            for qt in range(QT):
                v.tensor_scalar(eos[qt][:], eos[qt][:], 1.0, 0.0,
                                OP.subtract, OP.min)
                elu = workp.tile([128, NCLASS], fp32, tag=f"elu{qt}",
                                 name=f"elu{qt}")
                v.tensor_tensor(elu[:], os_[qt][:], eos[qt][:], OP.max)
                elus.append(elu)
            for qt in range(QT):
                se = workp.tile([128, 1], fp32, tag=f"se{qt}", name=f"se{qt}")
                e2 = workp.tile([128, NCLASS], fp32, tag="e2")
                sc.activation(e2[:], elus[qt][:], AF.Exp, accum_out=se[:])
                ses.append(se)
            lses = []
            for qt in range(QT):
                lse = workp.tile([128, 1], fp32, tag=f"lse{qt}",
                                 name=f"lse{qt}")
                sc.activation(lse[:], ses[qt][:], AF.Ln)
                lses.append(lse)
            for qt in range(QT):
                fin = workp.tile([128, NCLASS], fp32, tag="fin")
                v.tensor_scalar(fin[:], elus[qt][:], lses[qt][:], None,
                                OP.subtract)
                dma.dma_start(out[qt * 128:(qt + 1) * 128, :], fin[:])

    nc.finalize()
    return nc


def _get_compiled(no_cc=False, no_l1=False):
    key = ("nc", no_cc, no_l1)
    if key not in _CACHE:
        _CACHE[key] = _build_nc(no_cc=no_cc, no_l1=no_l1)
    return _CACHE[key]


def kernel(x, Wh, ah, Wo, ao):
    from concourse.bass_utils import run_bass_kernel_spmd

    nc = _get_compiled()
    x = np.asarray(x, np.float32)
    Wh = np.asarray(Wh, np.float32)
    ah = np.asarray(ah, np.float32)
    Wo = np.asarray(Wo, np.float32)
    ao = np.asarray(ao, np.float32)

    # host-side relayouts (no math): head-major weight matrix, block-diag
    # score matrix, split ao
    Whr = np.ascontiguousarray(
        Wh.transpose(1, 0, 2).reshape(NFEAT, HW))          # [512, 512]
    Asd = np.zeros((NFEAT, 16), np.float32)
    for h in range(NHEADS):
        Asd[h * NHID:(h + 1) * NHID, h] = ah[h, :NHID]      # src
        Asd[h * NHID:(h + 1) * NHID, 8 + h] = ah[h, NHID:]  # dst
    aod = np.stack([ao[:NCLASS], ao[NCLASS:]])              # [2, 16]

    in_maps = []
    for i in range(NC):
        in_maps.append({
            "xT": np.ascontiguousarray(x[i * NQ:(i + 1) * NQ].T),
            "Whr": Whr, "Asd": Asd,
            "Wo": np.ascontiguousarray(Wo), "aod": aod,
        })
    res = run_bass_kernel_spmd(nc, in_maps, list(range(NC)))
    return np.concatenate([res.results[i]["out"] for i in range(NC)], 0)

